# revision 8
# baseline (speedup 1.0000x reference)
"""Trainium2 Bass kernel for the CoLL co-occurrence layer.

Math (per image):
    scale = 8/(max(x)-min(x)+1e-8)   (global over the whole batch)
    u     = (x - xmin)*scale ;  idx = clip(floor(u), 0, 7)
    y(p)  = sum_q w[q] * x(p+q) * co[idx_p, idx_q]   over 3x3 neighborhoods q

Reformulation (staircase basis, select form):
    m_j    = x * 1[u >= j]           j = 1..7   (m_0 = x)
    n_i    = sum_j A[i,j] m_j        A = column-diffs of co  -> n_i = x * co[i, idx]
    V_i    = conv3x3(n_i, w)         (SAME, zero pad)
    y(p)   = V_{idx_p}(p)            via a chain of predicated copies

Mapping (one image per NeuronCore, batch 8 over 8 cores):
  - binning (u) is exact fp32, matching the reference op-for-op; the heavy
    path (masked fields, mix, conv) runs in bf16 (tolerance 2e-2).
  - m_j = (u >= j) * x in ONE scalar_tensor_tensor instruction per field; the
    m_j fields double as the predication masks for the final select (m_j != 0
    iff u >= j: x>0 everywhere except the global min, where all masks are 0).
    Field work is split between DVE and POOL (gpsimd).
  - layout [h=128 partitions, (w,c)=8192 free]; conv along h via banded-matrix
    matmuls on PE, conv along w via +-C free-dim reads; the 8x8 mix runs on PE
    with fields packed on partitions by stride-8 row groups (kron(I16, A^T)
    stationary); pack/unpack are SBUF-SBUF DMAs.
  - ACT evacuates mix PSUM -> bf16 SBUF and writes the V_0 select baseline;
    DVE runs the 7-deep copy_predicated select chain straight out of PSUM.
  - prologue: x DMA in halves with min (DVE) / max (POOL) partial reduces
    overlapped; partition-reduce via DVE 32x32 stream transpose; 2-float
    AllReduce(max) of (-min, max) for the scale.
"""

from contextlib import ExitStack

import numpy as np

import concourse.bass as bass
import concourse.tile as tile
from concourse import mybir
from concourse.tile_rust import add_dep_helper

F32 = mybir.dt.float32
BF16 = mybir.dt.bfloat16
U16 = mybir.dt.uint16
AX = mybir.AxisListType
OP = mybir.AluOpType

N, H, W, C = 8, 128, 128, 64
NB = 8
N_CORES = 8
Fd = W * C            # 8192
FC = 1024             # chunk width (output cols per chunk)
EXT = FC + 2 * C      # 1152: chunk + 64-halo each side
NCH = Fd // FC        # 8 chunks
POOL_PRODUCTS = (1, 2)   # field products run in-place on POOL; rest on DVE
POOL_MASKS = (1, 2, 3, 4, 5, 6, 7)  # is_ge masks computed on POOL


def build_tables(co, w):
    """Host-side weight-matrix construction from the tiny co/w inputs.

    mixW [128,128] bf16: kron(I16, A^T) with A = column-diffs of co, so that
      out[lo*8+i, c] = sum_j A[i,j] * in[li*8+j, c]  (li==lo).
    band [3,128,128] bf16: band[dw, hi, ho] = w[ho-hi+1, dw] (|ho-hi|<=1).
    """
    co = np.asarray(co, np.float32)
    w = np.asarray(w, np.float32)
    A = co - np.concatenate([np.zeros((NB, 1), np.float32), co[:, :-1]], axis=1)
    mixW = np.kron(np.eye(16, dtype=np.float32), A.T.copy())      # [128,128]
    band = np.zeros((3, 128, 128), np.float32)  # [dw, h_in, h_out]
    for dw in range(3):
        for ho in range(128):
            for dh in range(3):
                hi = ho + dh - 1
                if 0 <= hi < 128:
                    band[dw, hi, ho] = w[dh, dw]
    return {"mixW": mixW, "band": band}


def build_bass(n_cores=N_CORES, reps=1, loop_all=False):
    """Per-core Bass module; every core runs the same program on its own image
    (collective min/max when n_cores > 1). reps>1 wraps the main pipeline in a
    For_i for wall-clock HW timing; loop_all=True puts the whole program
    (input DMA, reduce, collective, chunks) inside the loop so the per-rep
    time approximates a full single-shot execution."""
    from concourse.bacc import Bacc
    nc = Bacc()
    x_d = nc.declare_dram_parameter("x", [H, Fd], F32, isOutput=False)
    mixW_d = nc.declare_dram_parameter("mixW", [128, 128], F32, isOutput=False)
    band_d = nc.declare_dram_parameter("band", [3, 128, 128], F32, isOutput=False)
    y_d = nc.declare_dram_parameter("y", [H, Fd], F32, isOutput=True)
    cc_in = nc.dram_tensor("cc_in", [2], F32)
    if n_cores > 1:
        cc_out = nc.dram_tensor("cc_out", [2], F32, addr_space="Shared")

    with tile.TileContext(nc) as tc, ExitStack() as ctx:
        consts = ctx.enter_context(tc.tile_pool(name="consts", bufs=1))
        upool = ctx.enter_context(tc.tile_pool(name="upool", bufs=2))
        mpool = ctx.enter_context(tc.tile_pool(name="mpool", bufs=2))
        ppool = ctx.enter_context(tc.tile_pool(name="ppool", bufs=2))
        mixps = ctx.enter_context(tc.tile_pool(name="mixps", bufs=2, space="PSUM"))
        mixpst = ctx.enter_context(tc.tile_pool(name="mixpst", bufs=1, space="PSUM"))
        npackp = ctx.enter_context(tc.tile_pool(name="npackp", bufs=2))
        nnatp = ctx.enter_context(tc.tile_pool(name="nnatp", bufs=2))
        convps = ctx.enter_context(tc.tile_pool(name="convps", bufs=3, space="PSUM"))
        ypool = ctx.enter_context(tc.tile_pool(name="ypool", bufs=2))

        # ---- constants: weights + fp32 zero-padded image, resident in SBUF ----
        mixWf = consts.tile([128, 128], F32)
        nc.sync.dma_start(out=mixWf, in_=mixW_d[:, :])
        mixW = consts.tile([128, 128], BF16)
        nc.vector.tensor_copy(mixW, mixWf)
        bandf = consts.tile([128, 3, 128], F32)
        nc.sync.dma_start(out=bandf, in_=band_d[:, :, :].rearrange("d i o -> i d o"))
        band = consts.tile([128, 3, 128], BF16)
        nc.vector.tensor_copy(band, bandf)

        xfp = consts.tile([128, Fd + 2 * C], F32)
        nc.gpsimd.memset(xfp[:, 0:C], 0.0)
        nc.gpsimd.memset(xfp[:, C + Fd:], 0.0)
        mnh = consts.tile([128, 2], F32)
        mxh = consts.tile([128, 2], F32)
        tmp32 = consts.tile([128, 32], F32)
        nc.gpsimd.memset(tmp32, -3e38)
        mn = consts.tile([128, 1], F32)
        tt = consts.tile([128, 32], F32)
        rr = consts.tile([128, 1], F32)
        s4 = consts.tile([2, 4], F32)
        red2 = consts.tile([2, 1], F32)
        pair = consts.tile([128, 2], F32)
        rng = consts.tile([128, 1], F32)
        dd = consts.tile([128, 1], F32)
        recip = consts.tile([128, 1], F32)
        scale = consts.tile([128, 1], F32)
        negxmin = pair[:, 0:1]
        gmax = pair[:, 1:2]
        HLF = Fd // 2

        def prologue():
            # x in two halves so the partial reduces overlap the DMA
            nc.sync.dma_start(out=xfp[:, C:C + HLF], in_=x_d[:, 0:HLF])
            nc.sync.dma_start(out=xfp[:, C + HLF:C + Fd], in_=x_d[:, HLF:Fd])

            # global min/max -> scale (exact fp32, matches reference)
            for hlf in range(2):
                win = xfp[:, C + hlf * HLF:C + (hlf + 1) * HLF]
                nc.vector.tensor_reduce(mnh[:, hlf:hlf + 1], win, axis=AX.X,
                                        op=OP.min)
                nc.vector.tensor_reduce(mxh[:, hlf:hlf + 1], win, axis=AX.X,
                                        op=OP.max)
            # tmp32 cols: 0 = -min, 1 = max, rest = -BIG filler
            nc.vector.tensor_reduce(mn, mnh, axis=AX.X, op=OP.min)
            nc.vector.tensor_scalar_mul(tmp32[:, 0:1], mn, -1.0)
            nc.vector.tensor_reduce(tmp32[:, 1:2], mxh, axis=AX.X, op=OP.max)
            # partition reduce via 32x32 block transpose: row 32b+r, col c of
            # the transposed tile holds tmp32[32b+c, r]
            nc.vector.transpose(tt, tmp32)
            nc.vector.tensor_reduce(rr, tt, axis=AX.X, op=OP.max)
            # rr[32b+0] = -min over block b, rr[32b+1] = max over block b
            nc.sync.dma_start(out=s4[0:1, :], in_=rr[0:128:32, 0:1])
            nc.sync.dma_start(out=s4[1:2, :], in_=rr[1:128:32, 0:1])
            nc.vector.tensor_reduce(red2, s4, axis=AX.X, op=OP.max)

            dma_in = nc.sync.dma_start(out=cc_in[:], in_=red2)
            src = cc_in
            prev = dma_in
            if n_cores > 1:
                cc = nc.gpsimd.collective_compute(
                    "AllReduce", OP.max,
                    replica_groups=[list(range(n_cores))],
                    ins=[cc_in.ap().opt()], outs=[cc_out.ap().opt()],
                )
                add_dep_helper(cc.ins, dma_in.ins, True, "cc waits dram write")
                src = cc_out
                prev = cc
            bcast = bass.AP(tensor=src.ap().tensor, offset=0,
                            ap=[[0, 128], [1, 2]])
            dma_back = nc.sync.dma_start(out=pair[:, :], in_=bcast)
            add_dep_helper(dma_back.ins, prev.ins, True, "bcast waits dram")

            nc.vector.tensor_tensor(rng, gmax, negxmin, op=OP.add)  # max-min
            nc.vector.tensor_scalar_add(dd, rng, float(np.float32(1e-8)))
            # scale = 8/d as 8*(1/d): exact wrt fl(8/d), *8 is a pow2 scale
            nc.vector.reciprocal(recip, dd)
            nc.vector.tensor_scalar_mul(scale, recip, 8.0)

        # ---- main streamed pipeline over free-dim chunks ----
        def chunk_pipeline(ci):
            cs = ci * FC  # data cols [cs-C, cs+FC+C) = xfp cols [cs, cs+EXT)
            xwin = xfp[:, cs:cs + EXT]
            u = upool.tile([128, EXT], F32, tag="u")
            nc.vector.tensor_scalar(u, xwin, negxmin, scale,
                                    op0=OP.add, op1=OP.mult)

            # masked fields, j-minor layout: mP[:, 0, :] = x, mP[:, j, :] =
            # (u >= j) * x. POOL writes the is_ge masks straight into the mP
            # slots, then the product runs in place (slot_j *= slot_0), split
            # across POOL and DVE. m_j doubles as the select mask later.
            mP = mpool.tile([128, NB, EXT], BF16, tag="mP")
            nc.gpsimd.tensor_copy(mP[:, 0, :], xwin)
            for j in range(1, NB):
                meng = nc.gpsimd if j in POOL_MASKS else nc.vector
                meng.tensor_scalar(mP[:, j, :], u, float(j), None, op0=OP.is_ge)
            for j in range(1, NB):
                peng = nc.gpsimd if j in POOL_PRODUCTS else nc.vector
                peng.tensor_tensor(mP[:, j, :], mP[:, j, :], mP[:, 0, :],
                                   op=OP.mult)

            # pack per stride-8 row group a (rows a, a+8, ..., a+120):
            # P[l*8+j, a, :] = mP[a+8l, j, :]
            P = ppool.tile([128, 8, EXT], BF16, tag="P")
            for a in range(8):
                nc.sync.dma_start(out=P[:, a, :], in_=mP[a:128:8, :, :])

            # mix: n[lo*8+i] = sum_j A[i,j] m_j  (kron(I16, A^T) stationary);
            # 128-col tails are batched 4 groups per matmul.
            npack = npackp.tile([128, 8, EXT], BF16, tag="npack")
            for s in range(8):
                pm = mixps.tile([128, 1024], F32, tag="pm")
                nc.tensor.matmul(pm[:, 0:512], mixW, P[:, s, 0:512],
                                 start=True, stop=True)
                nc.tensor.matmul(pm[:, 512:1024], mixW, P[:, s, 512:1024],
                                 start=True, stop=True)
                nc.scalar.copy(npack[:, s, 0:1024], pm)
                if s % 4 == 3:
                    pmt = mixpst.tile([128, 512], F32, tag="pmt")
                    nc.tensor.matmul(pmt, mixW, P[:, s - 3:s + 1, 1024:EXT],
                                     start=True, stop=True)
                    nc.scalar.copy(npack[:, s - 3:s + 1, 1024:EXT], pmt)

            # unpack per field i: nnat[8l+a, i, :] = npack[l*8+i, a, :]
            nnat = nnatp.tile([128, NB, EXT], BF16, tag="nnat")
            for i in range(NB):
                nc.sync.dma_start(out=nnat[:, i, :], in_=npack[i:128:8, :, :])

            # conv + select, per 512-col span
            for sp in range(0, FC, 512):
                y_t = ypool.tile([128, 512], F32, tag="y")
                for half in range(4):
                    vts = []
                    for k in range(2):
                        vt = convps.tile([128, 512], F32, tag="vt")
                        vts.append(vt)
                    for dw in range(3):
                        for k in range(2):
                            i = 2 * half + k
                            nc.tensor.matmul(
                                vts[k], band[:, dw, :],
                                nnat[:, i, sp + dw * C:sp + dw * C + 512],
                                start=(dw == 0), stop=(dw == 2))
                    for k in range(2):
                        i = 2 * half + k
                        if i == 0:
                            # V_0 baseline copy on ACT (frees DVE for preds)
                            nc.scalar.copy(y_t, vts[k])
                        else:
                            mask = mP[:, i, C + sp:C + sp + 512].bitcast(U16)
                            nc.vector.copy_predicated(y_t, mask, vts[k])
                nc.sync.dma_start(out=y_d[:, cs + sp:cs + sp + 512], in_=y_t)

        if reps == 1:
            prologue()
            for ci in range(NCH):
                chunk_pipeline(ci)
        elif loop_all:
            with tc.For_i(0, reps, 1):
                prologue()
                for ci in range(NCH):
                    chunk_pipeline(ci)
        else:
            prologue()
            with tc.For_i(0, reps, 1):
                for ci in range(NCH):
                    chunk_pipeline(ci)
    nc.finalize()
    return nc


_CACHE = {}


def _run(x, co_matrix, w_spatial, trace=False):
    x = np.ascontiguousarray(np.asarray(x, np.float32))
    tables = build_tables(co_matrix, w_spatial)
    n, h, w_, c = x.shape
    assert (n, h, w_, c) == (N, H, W, C), (n, h, w_, c)

    from concourse.bass_utils import run_bass_kernel_spmd

    key = "full"
    if key not in _CACHE:
        _CACHE[key] = build_bass(n_cores=N_CORES)
    nc = _CACHE[key]

    in_maps = []
    for core in range(N_CORES):
        in_maps.append({"x": x[core].reshape(H, W * C), **tables})
    res = run_bass_kernel_spmd(nc, in_maps, core_ids=list(range(N_CORES)),
                               trace=trace)
    out = np.stack([res.results[i]["y"].reshape(H, W, C) for i in range(N_CORES)], 0)
    return out, res


def kernel(x, co_matrix, w_spatial):
    return _run(x, co_matrix, w_spatial)[0]


def run_traced(x, co_matrix, w_spatial):
    _, res = _run(x, co_matrix, w_spatial, trace=True)
    return res.exec_time_ns


# revision 21
# speedup vs baseline: 3.7019x; 3.7019x over previous
"""Trainium2 Bass kernel for the CoLL co-occurrence layer.

Math (per image):
    scale = 8/(max(x)-min(x)+1e-8)   (global over the whole batch)
    u     = (x - xmin)*scale ;  idx = clip(floor(u), 0, 7)
    y(p)  = sum_q w[q] * x(p+q) * co[idx_p, idx_q]   over 3x3 neighborhoods q

Reformulation (staircase basis, select form):
    m_j    = x * 1[u >= j]           j = 1..7   (m_0 = x)
    n_i    = sum_j A[i,j] m_j        A = column-diffs of co  -> n_i = x * co[i, idx]
    V_i    = conv3x3(n_i, w)         (SAME, zero pad)
    y(p)   = V_{idx_p}(p)            via a chain of predicated copies

Mapping (one image per NeuronCore, batch 8 over 8 cores):
  - binning (u) is exact fp32, matching the reference op-for-op; the heavy
    path (masked fields, mix, conv) runs in bf16 (tolerance 2e-2).
  - m_j = (u >= j) * x in ONE scalar_tensor_tensor instruction per field; the
    m_j fields double as the predication masks for the final select (m_j != 0
    iff u >= j: x>0 everywhere except the global min, where all masks are 0).
    Field work is split between DVE and POOL (gpsimd).
  - layout [h=128 partitions, (w,c)=8192 free]; conv along h via banded-matrix
    matmuls on PE, conv along w via +-C free-dim reads; the 8x8 mix runs on PE
    with fields packed on partitions by stride-8 row groups (kron(I16, A^T)
    stationary); pack/unpack are SBUF-SBUF DMAs.
  - ACT evacuates mix PSUM -> bf16 SBUF and writes the V_0 select baseline;
    DVE runs the 7-deep copy_predicated select chain straight out of PSUM.
  - prologue: x DMA in halves with min (DVE) / max (POOL) partial reduces
    overlapped; partition-reduce via DVE 32x32 stream transpose; 2-float
    AllReduce(max) of (-min, max) for the scale.
"""

import os
from contextlib import ExitStack

import numpy as np

import concourse.bass as bass
import concourse.tile as tile
from concourse import mybir
from concourse.tile_rust import add_dep_helper

F32 = mybir.dt.float32
BF16 = mybir.dt.bfloat16
U16 = mybir.dt.uint16
AX = mybir.AxisListType
OP = mybir.AluOpType

N, H, W, C = 8, 128, 128, 64
NB = 8
N_CORES = 8
Fd = W * C            # 8192
FC = 1024             # chunk width (output cols per chunk)
EXT = FC + 2 * C      # 1152: chunk + 64-halo each side
NCH = Fd // FC        # 8 chunks

# experiment knobs (read once at import)
KNOB_M0 = os.environ.get("K_M0", "dve")          # dve | pool
KNOB_UNPACK = os.environ.get("K_UNPACK", "sync")  # sync | scalar
KNOB_Y = os.environ.get("K_Y", "sync")            # sync | gpsimd
# sim-only: replace partition-expanding pack/unpack sources with partition-
# matched slices (same descriptor count/shape/bytes, wrong data) so CoreSim's
# conservative AP-overlap check passes and the full kernel can be sim-timed.
KNOB_FAKEPACK = os.environ.get("K_FAKEPACK", "0") == "1"
KNOB_PRODPOOL = int(os.environ.get("K_PRODPOOL", "0"))  # N products on POOL
KNOB_STT = os.environ.get("K_STT", "0") == "1"    # fused scalar_tensor_tensor
KNOB_FLOORV = os.environ.get("K_FLOORV", "0") == "1"  # bf16 floor(u) masks



def build_tables(co, w):
    """Host-side weight-matrix construction from the tiny co/w inputs.

    mixW [128,128] bf16: kron(I16, A^T) with A = column-diffs of co, so that
      out[lo*8+i, c] = sum_j A[i,j] * in[li*8+j, c]  (li==lo).
    band [3,128,128] bf16: band[dw, hi, ho] = w[ho-hi+1, dw] (|ho-hi|<=1).
    """
    co = np.asarray(co, np.float32)
    w = np.asarray(w, np.float32)
    A = co - np.concatenate([np.zeros((NB, 1), np.float32), co[:, :-1]], axis=1)
    mixW = np.kron(np.eye(16, dtype=np.float32), A.T.copy())      # [128,128]
    band = np.zeros((3, 128, 128), np.float32)  # [dw, h_in, h_out]
    for dw in range(3):
        for ho in range(128):
            for dh in range(3):
                hi = ho + dh - 1
                if 0 <= hi < 128:
                    band[dw, hi, ho] = w[dh, dw]
    return {"mixW": mixW, "band": band}


def build_bass(n_cores=N_CORES, reps=1, loop_all=False):
    """Per-core Bass module; every core runs the same program on its own image
    (collective min/max when n_cores > 1). reps>1 wraps the main pipeline in a
    For_i for wall-clock HW timing; loop_all=True puts the whole program
    (input DMA, reduce, collective, chunks) inside the loop so the per-rep
    time approximates a full single-shot execution."""
    from concourse.bacc import Bacc
    nc = Bacc()
    x_d = nc.declare_dram_parameter("x", [H, Fd], F32, isOutput=False)
    mixW_d = nc.declare_dram_parameter("mixW", [128, 128], F32, isOutput=False)
    band_d = nc.declare_dram_parameter("band", [3, 128, 128], F32, isOutput=False)
    y_d = nc.declare_dram_parameter("y", [H, Fd], F32, isOutput=True)
    cc_in = nc.dram_tensor("cc_in", [2], F32)
    if n_cores > 1:
        cc_out = nc.dram_tensor("cc_out", [2], F32, addr_space="Shared")

    with tile.TileContext(nc) as tc, ExitStack() as ctx:
        consts = ctx.enter_context(tc.tile_pool(name="consts", bufs=1))
        upool = ctx.enter_context(tc.tile_pool(name="upool", bufs=2))
        mpool = ctx.enter_context(tc.tile_pool(name="mpool", bufs=2))
        ppool = ctx.enter_context(tc.tile_pool(name="ppool", bufs=2))
        mixps = ctx.enter_context(tc.tile_pool(name="mixps", bufs=3, space="PSUM"))
        mixpst = ctx.enter_context(tc.tile_pool(name="mixpst", bufs=1, space="PSUM"))
        npackp = ctx.enter_context(tc.tile_pool(name="npackp", bufs=2))
        nnatp = ctx.enter_context(tc.tile_pool(name="nnatp", bufs=2))
        convps = ctx.enter_context(tc.tile_pool(name="convps", bufs=4, space="PSUM"))
        ypool = ctx.enter_context(tc.tile_pool(name="ypool", bufs=2))

        # ---- constants: weights + fp32 zero-padded image, resident in SBUF ----
        mixWf = consts.tile([128, 128], F32)
        nc.sync.dma_start(out=mixWf, in_=mixW_d[:, :])
        mixW = consts.tile([128, 128], BF16)
        nc.vector.tensor_copy(mixW, mixWf)
        bandf = consts.tile([128, 3, 128], F32)
        nc.sync.dma_start(out=bandf, in_=band_d[:, :, :].rearrange("d i o -> i d o"))
        band = consts.tile([128, 3, 128], BF16)
        nc.vector.tensor_copy(band, bandf)

        xfp = consts.tile([128, Fd + 2 * C], F32)
        nc.gpsimd.memset(xfp[:, 0:C], 0.0)
        nc.gpsimd.memset(xfp[:, C + Fd:], 0.0)
        mnh = consts.tile([128, 2], F32)
        mxh = consts.tile([128, 2], F32)
        tmp32 = consts.tile([128, 32], F32)
        nc.gpsimd.memset(tmp32, -3e38)
        mn = consts.tile([128, 1], F32)
        tt = consts.tile([128, 32], F32)
        rr = consts.tile([128, 1], F32)
        s4 = consts.tile([2, 4], F32)
        red2 = consts.tile([2, 1], F32)
        pair = consts.tile([128, 2], F32)
        rng = consts.tile([128, 1], F32)
        dd = consts.tile([128, 1], F32)
        recip = consts.tile([128, 1], F32)
        scale = consts.tile([128, 1], F32)
        negxmin = pair[:, 0:1]
        gmax = pair[:, 1:2]
        HLF = Fd // 2

        def prologue():
            # x in two halves so the partial reduces overlap the DMA
            nc.sync.dma_start(out=xfp[:, C:C + HLF], in_=x_d[:, 0:HLF])
            nc.sync.dma_start(out=xfp[:, C + HLF:C + Fd], in_=x_d[:, HLF:Fd])

            # global min/max -> scale (exact fp32, matches reference)
            for hlf in range(2):
                win = xfp[:, C + hlf * HLF:C + (hlf + 1) * HLF]
                nc.vector.tensor_reduce(mnh[:, hlf:hlf + 1], win, axis=AX.X,
                                        op=OP.min)
                nc.vector.tensor_reduce(mxh[:, hlf:hlf + 1], win, axis=AX.X,
                                        op=OP.max)
            # tmp32 cols: 0 = -min, 1 = max, rest = -BIG filler
            nc.vector.tensor_reduce(mn, mnh, axis=AX.X, op=OP.min)
            nc.vector.tensor_scalar_mul(tmp32[:, 0:1], mn, -1.0)
            nc.vector.tensor_reduce(tmp32[:, 1:2], mxh, axis=AX.X, op=OP.max)
            # partition reduce via 32x32 block transpose: row 32b+r, col c of
            # the transposed tile holds tmp32[32b+c, r]
            nc.vector.transpose(tt, tmp32)
            nc.vector.tensor_reduce(rr, tt, axis=AX.X, op=OP.max)
            # rr[32b+0] = -min over block b, rr[32b+1] = max over block b
            rsl = (lambda o: rr[0:4, 0:1]) if KNOB_FAKEPACK else (
                lambda o: rr[o:128:32, 0:1])
            nc.sync.dma_start(out=s4[0:1, :], in_=rsl(0))
            nc.sync.dma_start(out=s4[1:2, :], in_=rsl(1))
            nc.vector.tensor_reduce(red2, s4, axis=AX.X, op=OP.max)

            dma_in = nc.sync.dma_start(out=cc_in[:], in_=red2)
            src = cc_in
            prev = dma_in
            if n_cores > 1:
                cc = nc.gpsimd.collective_compute(
                    "AllReduce", OP.max,
                    replica_groups=[list(range(n_cores))],
                    ins=[cc_in.ap().opt()], outs=[cc_out.ap().opt()],
                )
                add_dep_helper(cc.ins, dma_in.ins, True, "cc waits dram write")
                src = cc_out
                prev = cc
            bcast = bass.AP(tensor=src.ap().tensor, offset=0,
                            ap=[[0, 128], [1, 2]])
            dma_back = nc.sync.dma_start(out=pair[:, :], in_=bcast)
            add_dep_helper(dma_back.ins, prev.ins, True, "bcast waits dram")

            nc.vector.tensor_tensor(rng, gmax, negxmin, op=OP.add)  # max-min
            nc.vector.tensor_scalar_add(dd, rng, float(np.float32(1e-8)))
            # scale = 8/d as 8*(1/d): exact wrt fl(8/d), *8 is a pow2 scale
            nc.vector.reciprocal(recip, dd)
            nc.vector.tensor_scalar_mul(scale, recip, 8.0)

        # ---- main streamed pipeline over free-dim chunks ----
        # Superstep emission order (s = chunk index):
        #   fields(s) [DVE] -> pack(s) [SP] -> conv+V0+selects(s-1) [PE/ACT/
        #   DVE] -> mix+evac(s) [PE/ACT] -> unpack(s) [SP]
        # so each engine's in-order stream interleaves chunk s's front work
        # with chunk s-1's back work: V0(s-1) precedes evacs(s) on ACT, the
        # selects(s-1) follow fields(s) on DVE, conv(s-1) precedes mix(s) on
        # PE. The two 512-col select chains alternate so the serial RMW
        # chains on y_t hide each other's PSUM latency.
        def fields(ci):
            cs = ci * FC  # data cols [cs-C, cs+FC+C) = xfp cols [cs, cs+EXT)
            xwin = xfp[:, cs:cs + EXT]
            u = upool.tile([128, EXT], F32, tag="u")
            nc.vector.tensor_scalar(u, xwin, negxmin, scale,
                                    op0=OP.add, op1=OP.mult)
            # masked fields, j-minor: mP[:, 0, :] = x, mP[:, j, :] =
            # (u >= j) * x as is_ge mask then in-place product (all DVE: POOL
            # is ~5x slower + port-serializes; fused scalar_tensor_tensor
            # measures slower than the 2-op form). m_j doubles as the select
            # mask later (m_j != 0 iff u >= j; x == 0 only at the global
            # min where every mask is 0 too).
            mP = mpool.tile([128, NB, EXT], BF16, tag="mP")
            m0eng = nc.gpsimd if KNOB_M0 == "pool" else nc.vector
            m0eng.tensor_copy(mP[:, 0, :], xwin)
            if KNOB_STT:
                for j in range(1, NB):
                    nc.vector.scalar_tensor_tensor(
                        mP[:, j, :], u, float(j), xwin,
                        op0=OP.is_ge, op1=OP.mult)
                return mP
            if KNOB_FLOORV:
                vi = upool.tile([128, EXT], mybir.dt.int32, tag="vi")
                nc.vector.tensor_copy(vi, u)          # f32 -> i32 truncates
                vb = upool.tile([128, EXT], BF16, tag="vb")
                nc.vector.tensor_copy(vb, vi)         # exact: v in {0..7}
                msrc = vb
            else:
                msrc = u
            for j in range(1, NB):
                nc.vector.tensor_scalar(mP[:, j, :], msrc, float(j), None,
                                        op0=OP.is_ge)
            for j in range(1, NB):
                eng = nc.gpsimd if j <= KNOB_PRODPOOL else nc.vector
                eng.tensor_tensor(mP[:, j, :], mP[:, j, :], mP[:, 0, :],
                                  op=OP.mult)
            return mP

        def pack(ci, mP):
            # pack per stride-8 row group a: P[l*8+j, a, :] = mP[a+8l, j, :]
            P = ppool.tile([128, 8, EXT], BF16, tag="P")
            for a in range(8):
                psrc = mP[:, a, :] if KNOB_FAKEPACK else mP[a:128:8, :, :]
                nc.sync.dma_start(out=P[:, a, :], in_=psrc)
            return P

        def mixunpack(ci, P):
            # mix: n[lo*8+i] = sum_j A[i,j] m_j (kron(I16, A^T) stationary);
            # 128-col tails are batched 4 groups per matmul.
            npack = npackp.tile([128, 8, EXT], BF16, tag="npack")
            for s in range(8):
                pm = mixps.tile([128, 512], F32, tag="pm")
                nc.tensor.matmul(pm, mixW, P[:, s, 0:512],
                                 start=True, stop=True)
                nc.scalar.copy(npack[:, s, 0:512], pm)
                pm2 = mixps.tile([128, 512], F32, tag="pm")
                nc.tensor.matmul(pm2, mixW, P[:, s, 512:1024],
                                 start=True, stop=True)
                nc.scalar.copy(npack[:, s, 512:1024], pm2)
                if s % 4 == 3:
                    pmt = mixpst.tile([128, 512], F32, tag="pmt")
                    nc.tensor.matmul(pmt, mixW, P[:, s - 3:s + 1, 1024:EXT],
                                     start=True, stop=True)
                    nc.scalar.copy(npack[:, s - 3:s + 1, 1024:EXT], pmt)
            # unpack per field i: nnat[8l+a, i, :] = npack[l*8+i, a, :]
            nnat = nnatp.tile([128, NB, EXT], BF16, tag="nnat")
            unp = nc.scalar if KNOB_UNPACK == "scalar" else nc.sync
            for i in range(NB):
                usrc = npack[:, i, :] if KNOB_FAKEPACK else npack[i:128:8, :, :]
                unp.dma_start(out=nnat[:, i, :], in_=usrc)
            return nnat

        def convsel(ci, mP, nnat):
            cs = ci * FC
            y_t = ypool.tile([128, FC], F32, tag="y")
            for half in range(4):
                # 4 single-bank conv tiles: (field k) x (span sp)
                vts = [[None, None], [None, None]]
                for k in range(2):
                    for sp in range(2):
                        vt = convps.tile([128, 512], F32, tag="vt")
                        vts[k][sp] = vt
                for dw in range(3):          # LDWEIGHTS runs of 4
                    for k in range(2):
                        for sp in range(2):
                            i = 2 * half + k
                            base = sp * 512 + dw * C
                            nc.tensor.matmul(vts[k][sp], band[:, dw, :],
                                             nnat[:, i, base:base + 512],
                                             start=(dw == 0), stop=(dw == 2))
                for k in range(2):
                    i = 2 * half + k
                    for sp in range(2):      # spans interleaved: chains hide
                        ysl = y_t[:, sp * 512:(sp + 1) * 512]
                        if i == 0:
                            # V_0 baseline on ACT (emitted before evacs(s))
                            nc.scalar.copy(ysl, vts[k][sp])
                        else:
                            mask = mP[:, i, C + sp * 512:C + sp * 512 + 512]
                            nc.vector.copy_predicated(ysl, mask.bitcast(U16),
                                                      vts[k][sp])
            yeng = nc.gpsimd if KNOB_Y == "gpsimd" else nc.sync
            yeng.dma_start(out=y_d[:, cs:cs + FC], in_=y_t)

        def chunks():
            live = {}
            for s in range(NCH + 1):
                if s < NCH:
                    mP = fields(s)
                    P = pack(s, mP)
                if s >= 1:
                    convsel(s - 1, *live.pop(s - 1))
                if s < NCH:
                    nnat = mixunpack(s, P)
                    live[s] = (mP, nnat)

        if reps == 1:
            prologue()
            chunks()
        elif loop_all:
            with tc.For_i(0, reps, 1):
                prologue()
                chunks()
        else:
            prologue()
            with tc.For_i(0, reps, 1):
                chunks()
    nc.finalize()
    return nc


_CACHE = {}


def _run(x, co_matrix, w_spatial, trace=False):
    x = np.ascontiguousarray(np.asarray(x, np.float32))
    tables = build_tables(co_matrix, w_spatial)
    n, h, w_, c = x.shape
    assert (n, h, w_, c) == (N, H, W, C), (n, h, w_, c)

    from concourse.bass_utils import run_bass_kernel_spmd

    key = "full"
    if key not in _CACHE:
        _CACHE[key] = build_bass(n_cores=N_CORES)
    nc = _CACHE[key]

    in_maps = []
    for core in range(N_CORES):
        in_maps.append({"x": x[core].reshape(H, W * C), **tables})
    res = run_bass_kernel_spmd(nc, in_maps, core_ids=list(range(N_CORES)),
                               trace=trace)
    out = np.stack([res.results[i]["y"].reshape(H, W, C) for i in range(N_CORES)], 0)
    return out, res


def kernel(x, co_matrix, w_spatial):
    return _run(x, co_matrix, w_spatial)[0]


def run_traced(x, co_matrix, w_spatial):
    _, res = _run(x, co_matrix, w_spatial, trace=True)
    return res.exec_time_ns
